# revision 9
# baseline (speedup 1.0000x reference)
"""GumbelQuantizer Trainium2 kernel.

Forward pass of the reference collapses: soft - stop_gradient(soft) == 0, so
onehot == hard one-hot of argmax_o(logits + gumbel).  The kernel computes:
  g   = -ln(-ln(u + eps) + eps)
  y   = W_k @ x_chunk + b_k + g          (per codebook k)
  m_t = max_o y[o, t]
  onehot = exp(BIG * (y - m))            (exactly 1.0 at argmax, 0.0 elsewhere)
  quantized[d, t] = sum_o cb[o, d] * onehot[o, t]
  counts[o] = sum_t onehot[o, t]         (histogram -> perplexity on host)

Sharding: data-parallel over batch B=8 across the 8 cores; weights replicated.
"""

import functools

import numpy as np

B, DIM, T = 8, 1024, 2048
K, CS, CH = 8, 512, 128
OT = CS // 128          # 4 o-tiles of 128 codes
EPS = 1e-10
BIG = 1e9
NCORES = 8
HALF = T // 2           # 1024 (PSUM-bank sized chunks)

_LAST_EXEC_NS = [None]
_LAST_RESULTS = [None]


@functools.lru_cache(maxsize=1)
def _build():
    import concourse.bacc as bacc
    import concourse.mybir as mybir
    from concourse.tile import TileContext

    f32 = mybir.dt.float32
    AX = mybir.AxisListType.X
    Ln = mybir.ActivationFunctionType.Ln
    Exp = mybir.ActivationFunctionType.Exp

    nc = bacc.Bacc()

    xc_d = nc.dram_tensor("xc", [DIM, T], f32, kind="ExternalInput")
    noise_d = nc.dram_tensor("noise", [K, CS, T], f32, kind="ExternalInput")
    wT_d = nc.dram_tensor("wT", [128, K * OT * 128], f32, kind="ExternalInput")
    cb_d = nc.dram_tensor("cb", [128, K * OT * 128], f32, kind="ExternalInput")
    bias_d = nc.dram_tensor("bias", [128, K * OT], f32, kind="ExternalInput")
    ident_d = nc.dram_tensor("ident", [128, 128], f32, kind="ExternalInput")
    ones_d = nc.dram_tensor("ones", [1, 128], f32, kind="ExternalInput")

    quant_d = nc.dram_tensor("quant", [DIM, T], f32, kind="ExternalOutput")
    cnt_d = nc.dram_tensor("cnt", [128, K * OT], f32, kind="ExternalOutput")

    with TileContext(nc) as tc:
        with (
            tc.sbuf_pool(name="consts", bufs=1) as cpool,
            tc.sbuf_pool(name="unoise", bufs=2) as npool,
            tc.sbuf_pool(name="work", bufs=1) as wpool,
            tc.sbuf_pool(name="io", bufs=2) as iopool,
            tc.psum_pool(name="pl", bufs=2) as plpool,
            tc.psum_pool(name="pq", bufs=1) as pqpool,
            tc.psum_pool(name="pt", bufs=2) as ptpool,
        ):
            wT_sb = cpool.tile([128, K * OT * 128], f32)
            nc.sync.dma_start(wT_sb[:], wT_d[:])
            cb_sb = cpool.tile([128, K * OT * 128], f32)
            nc.sync.dma_start(cb_sb[:], cb_d[:])
            bias_sb = cpool.tile([128, K * OT], f32)
            nc.sync.dma_start(bias_sb[:], bias_d[:])
            ident_sb = cpool.tile([128, 128], f32)
            nc.sync.dma_start(ident_sb[:], ident_d[:])
            ones_sb = cpool.tile([1, 128], f32)
            nc.sync.dma_start(ones_sb[:], ones_d[:])
            cnt_sb = cpool.tile([128, K * OT], f32)
            epscol = cpool.tile([128, 1], f32)
            nc.vector.memset(epscol[:], EPS)

            for k in range(K):
                # ---- gumbel noise: U <- -(ln(-ln(u+eps)+eps) - b) -------
                U = npool.tile([128, OT * T], f32, tag="U")
                for ot in range(OT):
                    nc.gpsimd.dma_start(
                        U[:, ot * T : (ot + 1) * T],
                        noise_d[k, ot * 128 : (ot + 1) * 128, :],
                    )
                for ot in range(OT):
                    usl = U[:, ot * T : (ot + 1) * T]
                    nc.scalar.activation(usl, usl, Ln, bias=epscol[:], scale=1.0)
                    nc.scalar.activation(usl, usl, Ln, bias=epscol[:], scale=-1.0)
                # H2 = H - b  (so y = l - H2 = l + b - H = l + b + g)
                for ot in range(OT):
                    nc.vector.tensor_scalar_sub(
                        U[:, ot * T : (ot + 1) * T],
                        U[:, ot * T : (ot + 1) * T],
                        bias_sb[:, k * OT + ot : k * OT + ot + 1],
                    )

                xk = iopool.tile([128, T], f32, tag="xk")
                nc.sync.dma_start(xk[:], xc_d[k * 128 : (k + 1) * 128, :])

                # ---- logits & y = l - H2 (y overwrites U in place) ------
                for ot in range(OT):
                    wcol = (k * OT + ot) * 128
                    for h in range(2):
                        pl = plpool.tile([128, HALF], f32, tag="pl")
                        for w in range(2):
                            nc.tensor.matmul(
                                pl[:, w * 512 : (w + 1) * 512],
                                wT_sb[:, wcol : wcol + 128],
                                xk[:, h * HALF + w * 512 : h * HALF + (w + 1) * 512],
                                start=True,
                                stop=True,
                            )
                        ysl = U[:, ot * T + h * HALF : ot * T + (h + 1) * HALF]
                        nc.vector.tensor_sub(ysl, pl[:], ysl)

                # ---- m = max over all 512 codes, per t ------------------
                m4 = wpool.tile([128, T], f32, tag="m4")
                nc.vector.tensor_max(m4[:], U[:, 0:T], U[:, T : 2 * T])
                nc.vector.tensor_max(m4[:], m4[:], U[:, 2 * T : 3 * T])
                nc.vector.tensor_max(m4[:], m4[:], U[:, 3 * T : 4 * T])
                mcol = wpool.tile([128, 16], f32, tag="mcol")
                for j in range(16):
                    pt = ptpool.tile([128, 128], f32, tag="pt")
                    nc.tensor.transpose(pt[:], m4[:, j * 128 : (j + 1) * 128], ident_sb[:])
                    nc.vector.reduce_max(mcol[:, j : j + 1], pt[:], axis=AX)
                negc = wpool.tile([128, 16], f32, tag="negc")
                nc.vector.tensor_scalar_mul(negc[:], mcol[:], -1.0)
                pmt = ptpool.tile([16, 128], f32, tag="pt")
                nc.tensor.transpose(pmt[:], negc[:], ident_sb[:])
                negm16 = wpool.tile([16, 128], f32, tag="negm16")
                nc.vector.tensor_copy(negm16[:], pmt[:])
                negmrow = wpool.tile([1, T], f32, tag="negmrow")
                nc.sync.dma_start(negmrow[0:1, :], negm16[:, :])

                # ---- z = y - m in PSUM; onehot = exp(BIG*z) + counts ----
                OH = wpool.tile([128, OT * T], f32, tag="OH")
                acc = wpool.tile([128, 2 * OT], f32, tag="acc")
                for ot in range(OT):
                    for h in range(2):
                        pz = plpool.tile([128, HALF], f32, tag="pl")
                        for w in range(2):
                            nc.tensor.matmul(
                                pz[:, w * 512 : (w + 1) * 512],
                                ident_sb[:],
                                U[:, ot * T + h * HALF + w * 512 : ot * T + h * HALF + (w + 1) * 512],
                                start=True,
                                stop=False,
                                skip_group_check=True,
                            )
                        for w in range(2):
                            nc.tensor.matmul(
                                pz[:, w * 512 : (w + 1) * 512],
                                ones_sb[:],
                                negmrow[0:1, h * HALF + w * 512 : h * HALF + (w + 1) * 512],
                                start=False,
                                stop=True,
                                skip_group_check=True,
                            )
                        nc.scalar.activation(
                            OH[:, ot * T + h * HALF : ot * T + (h + 1) * HALF],
                            pz[:],
                            Exp,
                            scale=BIG,
                            accum_out=acc[:, ot * 2 + h : ot * 2 + h + 1],
                        )
                    nc.vector.tensor_add(
                        cnt_sb[:, k * OT + ot : k * OT + ot + 1],
                        acc[:, ot * 2 : ot * 2 + 1],
                        acc[:, ot * 2 + 1 : ot * 2 + 2],
                    )

                # ---- quantized = cb.T @ onehot --------------------------
                for h in range(2):
                    pq = pqpool.tile([128, HALF], f32, tag="pq")
                    for ot in range(OT):
                        ccol = (k * OT + ot) * 128
                        for w in range(2):
                            nc.tensor.matmul(
                                pq[:, w * 512 : (w + 1) * 512],
                                cb_sb[:, ccol : ccol + 128],
                                OH[:, ot * T + h * HALF + w * 512 : ot * T + h * HALF + (w + 1) * 512],
                                start=(ot == 0),
                                stop=(ot == OT - 1),
                            )
                    outq = iopool.tile([128, HALF], f32, tag="outq")
                    nc.vector.tensor_copy(outq[:], pq[:])
                    nc.sync.dma_start(
                        quant_d[k * 128 : (k + 1) * 128, h * HALF : (h + 1) * HALF],
                        outq[:],
                    )

            nc.sync.dma_start(cnt_d[:], cnt_sb[:])

    nc.compile()
    return nc


@functools.lru_cache(maxsize=1)
def _host_consts():
    # wT[c, (k*OT+ot)*128 + o] = proj_w[k, ot*128+o, c]  etc. are built in
    # kernel() from the actual inputs; here only the static ones.
    ident = np.eye(128, dtype=np.float32)
    ones = np.ones((1, 128), dtype=np.float32)
    return ident, ones


def kernel(x, proj_w, proj_b, codebooks, uniform_noise):
    from concourse.bass_utils import run_bass_kernel_spmd

    x = np.ascontiguousarray(np.asarray(x, dtype=np.float32))
    proj_w = np.asarray(proj_w, dtype=np.float32)
    proj_b = np.asarray(proj_b, dtype=np.float32)
    codebooks = np.asarray(codebooks, dtype=np.float32)
    uniform_noise = np.ascontiguousarray(np.asarray(uniform_noise, dtype=np.float32))

    # weight prep (tiny): lhsT layouts
    # wT [128c, K*OT*128]: column (k*OT+ot)*128+o  = proj_w[k, ot*128+o, c]
    wT = np.transpose(proj_w.reshape(K, OT, 128, CH), (3, 0, 1, 2)).reshape(CH, K * OT * 128)
    wT = np.ascontiguousarray(wT)
    # cb [128o, K*OT*128]: column (k*OT+ot)*128+d = codebooks[k, ot*128+o, d]
    cb = np.transpose(codebooks.reshape(K, OT, 128, CH), (2, 0, 1, 3)).reshape(128, K * OT * CH)
    cb = np.ascontiguousarray(cb)
    # bias [128p, K*OT]: bias[p, k*OT+ot] = proj_b[k, ot*128+p]
    bias = np.ascontiguousarray(np.transpose(proj_b.reshape(K, OT, 128), (2, 0, 1)).reshape(128, K * OT))
    ident, ones = _host_consts()

    nc = _build()
    in_maps = []
    for b in range(NCORES):
        in_maps.append(
            {
                "xc": x[b],
                "noise": uniform_noise[b],
                "wT": wT,
                "cb": cb,
                "bias": bias,
                "ident": ident,
                "ones": ones,
            }
        )

    res = run_bass_kernel_spmd(nc, in_maps, core_ids=list(range(NCORES)))
    _LAST_EXEC_NS[0] = res.exec_time_ns
    _LAST_RESULTS[0] = res

    quant = np.stack([res.results[b]["quant"] for b in range(NCORES)], axis=0)

    # counts[k, ot*128+p] = sum_cores cnt[p, k*OT+ot]
    cnt = np.zeros((128, K * OT), dtype=np.float64)
    for b in range(NCORES):
        cnt += res.results[b]["cnt"].astype(np.float64)
    counts = np.transpose(cnt.reshape(128, K, OT), (1, 2, 0)).reshape(K, CS)

    avg = (counts / float(B * T)).astype(np.float32)
    ent = -avg * np.log(avg + np.float32(1e-8))
    perplexity = np.exp(np.nansum(ent, axis=1, dtype=np.float32)).astype(np.float32)

    return quant, perplexity
